# revision 13
# baseline (speedup 1.0000x reference)
"""GNN message-passing (2-layer conv + log_softmax) as a Bass/Tile SPMD kernel
on 8 Trainium2 NeuronCores.

Strategy (v2 — batched dma_gather + selection-matrix matmul reduce):
  - nodes dst-sharded 8-way; tables (h1, t2) replicated via chunked AllGather
  - per core the edge stream is sorted (chunk, dst-tile, 32-dst segment) and
    padded to 128-edge columns; one dma_gather per ~64 columns fetches the
    per-edge message rows (ant ucode: ~0.34ns/descriptor vs 8.6ns/row for
    per-slot indirect DMA)
  - segment-sum on the TensorEngine: per column a host-built selection matrix
    S [128 edges, 32 dsts] (bf16 0/1) and matmul accumulate into PSUM;
    per (tile, chunk) partials are DVE-added into an SBUF accumulator
  - elu folded as g' = relu(f) + exp(min(f,0)) = elu(f)+1, compensated by
    b2' = b2 - W2.sum(0); log_softmax fused on ACT/DVE with a single batched
    Ln over all tiles' sum-exp values.
  - tables are 128 bf16 wide (256B rows; dma_gather elem constraint); int16
    gather indices are chunk-local (each chunk <= 32767 rows incl. zero row)
"""

import os
import sys

sys.path.insert(0, "/opt/trn_rl_repo")

import numpy as np
import ml_dtypes

BF16 = ml_dtypes.bfloat16

N_CORES = 8
P = 128
SEG = 32          # dsts per selection-matrix segment
NSEG = P // SEG   # segments per dst tile
CALLCOLS = 64     # max 128-edge columns per dma_gather call (num_idxs <= 8192)
TW = 128          # table row width (elements, bf16) -> 256B rows


def _make_cfg(n_nodes, n_edges, f_in=512, hid=64, n_cls=40):
    np_ = n_nodes // N_CORES
    assert np_ * N_CORES == n_nodes
    nw = (np_ + P - 1) // P
    npad = nw * P
    n_chunks = min(4, nw)
    tiles = [nw // n_chunks + (1 if i < nw % n_chunks else 0) for i in range(n_chunks)]
    tstart = np.concatenate([[0], np.cumsum(tiles)]).astype(int)
    cs = [int(tstart[i] * P) for i in range(n_chunks + 1)]  # padded node ranges
    cz = [cs[i + 1] - cs[i] for i in range(n_chunks)]
    crows = [1 + N_CORES * z for z in cz]                   # +1 zero row per chunk
    cb = np.concatenate([[0], np.cumsum(crows)]).astype(int)
    assert max(crows) <= 32767
    return dict(
        N=n_nodes, E=n_edges, F=f_in, H=hid, C=n_cls, NP=np_, NW=nw, NPAD=npad,
        NCH=n_chunks, TILES=tiles, TSTART=tstart, CS=cs, CZ=cz,
        CROWS=crows, CB=cb, TOT=int(cb[-1]),
    )


FULL_CFG = _make_cfg(100000, 3200000)


# ---------------------------------------------------------------- host prep

def host_prep(cfg, x, edge_index, W1, b1, W2, b2):
    N, NP, NW, NCH = cfg["N"], cfg["NP"], cfg["NW"], cfg["NCH"]
    F, H, C = cfg["F"], cfg["H"], cfg["C"]
    CS, CZ, CB = cfg["CS"], cfg["CZ"], cfg["CB"]
    src = np.asarray(edge_index[0]).astype(np.int64)
    dst = np.asarray(edge_index[1]).astype(np.int64)

    # global node id -> (chunk, chunk-local table row)
    r = np.arange(N, dtype=np.int64) // NP
    l = np.arange(N, dtype=np.int64) % NP
    csb = np.asarray(CS)
    ch_of = np.searchsorted(csb, l, side="right") - 1
    czv = np.asarray(CZ + [1])[ch_of]
    locrow = 1 + r * czv + (l - csb[ch_of])

    per_core = []
    for k in range(N_CORES):
        sel = (dst >= k * NP) & (dst < (k + 1) * NP)
        s_k = src[sel]
        d_k = dst[sel] - k * NP
        # sort edges by (chunk(src), dst)
        key = ch_of[s_k] * (NP + 1) + d_k
        order = np.argsort(key, kind="stable")
        s_k, d_k = s_k[order], d_k[order]
        ch_k = ch_of[s_k]
        lr_k = locrow[s_k]

        # build column stream per chunk: within chunk, per (tile, seg) pad
        # edge count to a multiple of 128
        idx_stream = []     # chunk-local int16 rows, len = cols*128 per chunk
        s_rows = []         # per column: [128, SEG] bf16 selection
        colmeta = []        # per chunk: list of (tile, seg, start, stop, tc_last)
        ncols_ch = []
        for c in range(NCH):
            m = ch_k == c
            dc = d_k[m]
            lrc = lr_k[m]
            idx_c = []
            meta_c = []
            # group by (tile, seg)
            gkey = dc // SEG
            # boundaries of equal gkey runs
            if len(dc):
                bnd = np.flatnonzero(np.diff(gkey)) + 1
                starts = np.concatenate([[0], bnd, [len(dc)]])
            else:
                starts = np.array([0])
            seg_cols = {}
            for bi in range(len(starts) - 1):
                a, b = int(starts[bi]), int(starts[bi + 1])
                g = int(gkey[a])
                t, q = g // NSEG, g % NSEG
                cnt = b - a
                ncol = (cnt + P - 1) // P
                rows = np.zeros(ncol * P, dtype=np.int16)
                rows[:cnt] = lrc[a:b]
                dloc = np.full(ncol * P, -1, dtype=np.int64)
                dloc[:cnt] = dc[a:b] - g * SEG
                for j in range(ncol):
                    sm = np.zeros((P, SEG), dtype=BF16)
                    dj = dloc[j * P:(j + 1) * P]
                    val = dj >= 0
                    sm[np.arange(P)[val], dj[val]] = 1.0
                    s_rows.append(sm)
                    idx_c.append(rows[j * P:(j + 1) * P])
                    meta_c.append([t, q, j == 0, j == ncol - 1, False])
                seg_cols[g] = ncol
            # mark last column of each (tile, chunk)
            last_t = {}
            for ci, mm in enumerate(meta_c):
                last_t[mm[0]] = ci
            for t, ci in last_t.items():
                meta_c[ci][4] = True
            colmeta.append(meta_c)
            ncols_ch.append(len(meta_c))
            idx_stream.append(
                np.concatenate(idx_c) if idx_c else np.zeros(0, np.int16)
            )

        # pack indices: per chunk, int16 [128, cols*8] (16-wrap, 8x replicated)
        packed = []
        for c in range(NCH):
            st = idx_stream[c]
            ncol = ncols_ch[c]
            if ncol == 0:
                continue
            t16 = st.reshape(ncol * 8, 16).T  # [16, ncol*8]
            packed.append(np.tile(t16, (8, 1)))
        idxp = (
            np.concatenate(packed, axis=1)
            if packed else np.zeros((P, 1), np.int16)
        )
        per_core.append(dict(
            colmeta=colmeta, ncols_ch=ncols_ch, idxp=np.ascontiguousarray(idxp),
        ))

    # schedules must be identical across cores for SPMD: use per-core maxima?
    # -> No: build one program per... SPMD needs ONE program. Pad every core's
    # schedule to the max column structure: instead, rebuild with a COMMON
    # column layout: for each (chunk, tile, seg) use the max column count
    # across cores.
    maxcols = {}
    for k in range(N_CORES):
        for c in range(NCH):
            cnt = {}
            for t, q, st, sp, tl in per_core[k]["colmeta"][c]:
                cnt[(t, q)] = cnt.get((t, q), 0) + 1
            for key2, v in cnt.items():
                maxcols[(c,) + key2] = max(maxcols.get((c,) + key2, 0), v)

    # rebuild per-core streams on the common layout
    common_meta = []  # per chunk: list of (tile, seg, start, stop, tc_last)
    for c in range(NCH):
        meta_c = []
        keys = sorted(k2 for k2 in maxcols if k2[0] == c)
        for (_, t, q) in keys:
            ncol = maxcols[(c, t, q)]
            for j in range(ncol):
                meta_c.append([t, q, j == 0, j == ncol - 1, False])
        last_t = {}
        for ci, mm in enumerate(meta_c):
            last_t[mm[0]] = ci
        for t, ci in last_t.items():
            meta_c[ci][4] = True
        common_meta.append(meta_c)

    s_all = []   # [cols_total][128, SEG] selection, shared layout, per core
    idxp_all = []
    for k in range(N_CORES):
        sel = (dst >= k * NP) & (dst < (k + 1) * NP)
        s_k = src[sel]
        d_k = dst[sel] - k * NP
        key = ch_of[s_k] * (NP + 1) + d_k
        order = np.argsort(key, kind="stable")
        s_k, d_k = s_k[order], d_k[order]
        ch_k = ch_of[s_k]
        lr_k = locrow[s_k]
        s_list, idx_list = [], []
        for c in range(NCH):
            m = ch_k == c
            dc = d_k[m]
            lrc = lr_k[m]
            gkey = dc // SEG
            pos = 0
            for (cc, t, q) in sorted(k2 for k2 in maxcols if k2[0] == c):
                g = t * NSEG + q
                mm = gkey == g
                cnt = int(mm.sum())
                ncol = maxcols[(c, t, q)]
                rows = np.zeros(ncol * P, dtype=np.int16)
                rows[:cnt] = lrc[mm]
                dloc = np.full(ncol * P, -1, dtype=np.int64)
                dloc[:cnt] = dc[mm] - g * SEG
                for j in range(ncol):
                    sm = np.zeros((P, SEG), dtype=BF16)
                    dj = dloc[j * P:(j + 1) * P]
                    val = dj >= 0
                    sm[np.arange(P)[val], dj[val]] = 1.0
                    s_list.append(sm)
                    idx_list.append(rows[j * P:(j + 1) * P])
        # pack
        st = np.concatenate(idx_list) if idx_list else np.zeros(16, np.int16)
        ncol_t = len(idx_list)
        t16 = st.reshape(max(ncol_t, 1) * 8, 16).T
        idxp_all.append(np.ascontiguousarray(np.tile(t16, (8, 1))))
        s_all.append(np.ascontiguousarray(
            np.stack(s_list, axis=1).reshape(P, -1) if s_list
            else np.zeros((P, SEG), BF16)
        ))

    # per-core tensors
    W1b = np.asarray(W1, dtype=np.float32).astype(BF16)
    W2b = np.asarray(W2, dtype=np.float32).astype(BF16)
    b1r = np.tile(np.asarray(b1, dtype=np.float32)[None, :], (P, 1))
    b2a = np.asarray(b2, dtype=np.float32) - np.asarray(W2, np.float32).sum(0)
    b2r = np.tile(b2a[None, :], (P, 1))
    in_maps = []
    xf = np.asarray(x, dtype=np.float32)
    for k in range(N_CORES):
        xT = np.ascontiguousarray(xf[k * NP:(k + 1) * NP].T).astype(BF16)
        in_maps.append(dict(
            xT=xT, W1=W1b, b1r=b1r, W2=W2b, b2r=b2r,
            idxp=idxp_all[k], smat=s_all[k],
        ))
    sched = dict(common_meta=common_meta)
    return sched, in_maps


# ---------------------------------------------------------------- device code

def _dma_gather_raw(gp, out_ap, in_ap, idxs_ap, num_idxs, elem_size, elem_step):
    """bass.dma_gather minus the elem_size%256 assert (non-transpose, HBM src),
    single_packet=False."""
    import concourse.mybir as mybir
    from concourse import ap_utils

    assert idxs_ap.dtype == mybir.dt.int16
    assert in_ap.dtype == out_ap.dtype
    assert ap_utils.ap_is_contiguous(out_ap.ap[1:])
    assert ap_utils.ap_is_contiguous(idxs_ap.ap[1:])
    assert in_ap.ap[0][0] == elem_step
    stride_bytes = elem_step * mybir.dt.size(in_ap.dtype)
    stride_bytes_256 = stride_bytes // 256
    assert stride_bytes_256 * 256 == stride_bytes and stride_bytes_256 < 256
    _in_ap = gp.lower_ap_dma(in_ap, for_custom_bir_dma=True)
    _idxs_ap = gp.lower_ap(idxs_ap)
    _out_ap = gp.lower_ap(out_ap)
    return gp.add_instruction(
        mybir.InstDMAGatherAnt(
            name=gp.bass.get_next_instruction_name(),
            ins=[*_in_ap, _idxs_ap, gp.lower_val_access(gp.to_reg(num_idxs))],
            outs=[_out_ap],
            transpose=False,
            num_idxs=num_idxs,
            elem_size=elem_size,
            stride_bytes_256=stride_bytes_256,
            gen_mode=0,
            single_packet=False,
            queue_num=0,
            sbuf_tokens_per_rank=0,
            sbuf_free_dim_per_rank=0,
            sbuf_free_dim_pad_per_rank=0,
            sbuf_byte_offset=0,
        )
    )


def build_program(cfg, sched, elem1=TW, elem2=TW):
    import concourse.bass as bass
    import concourse.bacc as bacc
    import concourse.mybir as mybir
    from concourse.tile import TileContext
    from concourse.masks import make_identity

    dt = mybir.dt
    N, F, H, C = cfg["N"], cfg["F"], cfg["H"], cfg["C"]
    NP, NW, NPAD, NCH = cfg["NP"], cfg["NW"], cfg["NPAD"], cfg["NCH"]
    CS, CZ, CB, CROWS = cfg["CS"], cfg["CZ"], cfg["CB"], cfg["CROWS"]
    TOT = cfg["TOT"]
    meta = sched["common_meta"]
    ncols_ch = [len(m) for m in meta]
    totcols = sum(ncols_ch)
    KF = F // P

    nc = bacc.Bacc(
        "TRN2", target_bir_lowering=False, debug=False, num_devices=N_CORES
    )
    xT = nc.declare_dram_parameter("xT", [F, NP], dt.bfloat16, isOutput=False)
    W1p = nc.declare_dram_parameter("W1", [F, H], dt.bfloat16, isOutput=False)
    b1p = nc.declare_dram_parameter("b1r", [P, H], dt.float32, isOutput=False)
    W2p = nc.declare_dram_parameter("W2", [H, C], dt.bfloat16, isOutput=False)
    b2p = nc.declare_dram_parameter("b2r", [P, C], dt.float32, isOutput=False)
    ixp = nc.declare_dram_parameter(
        "idxp", [P, max(totcols, 1) * 8], dt.int16, isOutput=False
    )
    smp = nc.declare_dram_parameter(
        "smat", [P, max(totcols, 1) * SEG], dt.bfloat16, isOutput=False
    )
    outp = nc.declare_dram_parameter("out", [NPAD, C], dt.float32, isOutput=True)

    rg = [list(range(N_CORES))]

    # split each chunk's columns into gather calls of <= CALLCOLS
    calls = []  # (chunk, col_start(global), ncols)
    goff = 0
    for c in range(NCH):
        nc_c = ncols_ch[c]
        o = 0
        while o < nc_c:
            n = min(CALLCOLS, nc_c - o)
            calls.append((c, goff + o, n))
            o += n
        goff += nc_c

    with TileContext(nc) as tc:
        with (
            tc.tile_pool(name="const", bufs=1) as const,
            tc.tile_pool(name="dram", bufs=1, space="DRAM") as dram,
            tc.tile_pool(name="xp", bufs=3) as xp,
            tc.tile_pool(name="hb", bufs=2) as hb,
            tc.tile_pool(name="ixp", bufs=2) as ixpool,
            tc.tile_pool(name="smp", bufs=2) as smpool,
            tc.tile_pool(name="gp", bufs=2) as gpl,
            tc.tile_pool(name="acc", bufs=1) as accp,
            tc.tile_pool(name="sp", bufs=4) as sp,
            tc.tile_pool(name="ps", bufs=6, space="PSUM") as ps,
        ):
            # --- constants
            w1sb = const.tile([P, KF, H], dt.bfloat16)
            nc.sync.dma_start(out=w1sb[:], in_=W1p[:].rearrange("(c p) h -> p c h", p=P))
            w2sb = const.tile([H, C], dt.bfloat16)
            nc.sync.dma_start(out=w2sb[:], in_=W2p[:])
            b1sb = const.tile([P, H], dt.float32)
            nc.sync.dma_start(out=b1sb[:], in_=b1p[:])
            b2sb = const.tile([P, C], dt.float32)
            nc.sync.dma_start(out=b2sb[:], in_=b2p[:])
            ident = const.tile([P, P], dt.bfloat16)
            make_identity(nc, ident[:])

            # --- internal DRAM
            h1k = dram.tile([NPAD, TW], dt.bfloat16)
            t2k = dram.tile([NPAD, TW], dt.bfloat16)
            tb1 = dram.tile([TOT, TW], dt.bfloat16)
            tb2 = dram.tile([TOT, TW], dt.bfloat16)

            zt = const.tile([1, TW], dt.bfloat16)
            nc.gpsimd.memset(zt[:], 0.0)
            for c in range(NCH):
                nc.sync.dma_start(out=tb1[CB[c]:CB[c] + 1, :], in_=zt[:])
                nc.sync.dma_start(out=tb2[CB[c]:CB[c] + 1, :], in_=zt[:])

            # --- accumulators / output staging
            acc1 = accp.tile([P, NW, H], dt.float32)
            acc2 = accp.tile([P, NW, C], dt.float32)
            outb = accp.tile([P, NW, C], dt.float32)
            ssum = accp.tile([P, NW], dt.float32)
            lsm = accp.tile([P, NW], dt.float32)

            # --- phase 1: h1 = x@W1 + b1, chunked AllGather
            xTr = xT[:].rearrange("(c p) n -> p c n", p=P)
            for c in range(NCH):
                t0, t1 = int(cfg["TSTART"][c]), int(cfg["TSTART"][c + 1])
                ntl = t1 - t0
                h1b = hb.tile([P, ntl, TW], dt.bfloat16, tag="h1b")
                nc.vector.memset(h1b[:].rearrange("p a b -> p (a b)"), 0.0)
                for i, nt in enumerate(range(t0, t1)):
                    cs = min(P, NP - nt * P)
                    if cs <= 0:
                        continue
                    xt = xp.tile([P, KF, P], dt.bfloat16, tag="xt")
                    nc.sync.dma_start(out=xt[:, :, :cs], in_=xTr[:, :, nt * P:nt * P + cs])
                    ph = ps.tile([P, H], dt.float32, tag="ph", bufs=2)
                    for kf in range(KF):
                        nc.tensor.matmul(
                            out=ph[:cs, :], lhsT=xt[:, kf, :cs], rhs=w1sb[:, kf, :],
                            start=(kf == 0), stop=(kf == KF - 1),
                        )
                    nc.vector.tensor_tensor(
                        out=h1b[:cs, i, :H], in0=ph[:cs, :], in1=b1sb[:cs, :],
                        op=mybir.AluOpType.add,
                    )
                nc.sync.dma_start(
                    out=h1k[CS[c]:CS[c] + ntl * P, :].rearrange(
                        "(a p) w -> p a w", p=P),
                    in_=h1b[:],
                )
                nc.gpsimd.collective_compute(
                    "AllGather", mybir.AluOpType.bypass, replica_groups=rg,
                    ins=[h1k[CS[c]:CS[c] + CZ[c], :]],
                    outs=[tb1[CB[c] + 1:CB[c] + 1 + N_CORES * CZ[c], :]],
                )

            # how many chunks contribute to each tile (common layout)
            tile_nch = [0] * NW
            for c in range(NCH):
                seen = set()
                for (t, q, st, sp_, tl) in meta[c]:
                    seen.add(t)
                for t in seen:
                    tile_nch[t] += 1

            # --- aggregation pass helper
            def agg_pass(tbl, elem, width, acc, post_tile):
                """gather+reduce all chunks into acc [P, NW, width]; call
                post_tile(t) after tile t's last chunk contribution."""
                pend = {}   # tile -> psum tile
                first = {}  # tile -> number of chunks folded in
                for (c, g0, ncols) in calls:
                    NI = ncols * P
                    ixt = ixpool.tile([P, ncols * 8], dt.int16, tag="ix")
                    nc.sync.dma_start(out=ixt[:], in_=ixp[:, g0 * 8:(g0 + ncols) * 8])
                    smt = smpool.tile([P, ncols, SEG], dt.bfloat16, tag="sm")
                    nc.sync.dma_start(
                        out=smt[:],
                        in_=smp[:, g0 * SEG:(g0 + ncols) * SEG].rearrange(
                            "p (n s) -> p n s", s=SEG),
                    )
                    gt = gpl.tile([P, ncols, elem], dt.bfloat16, tag="gt")
                    src_ap = tbl[CB[c]:CB[c] + CROWS[c], :elem] if elem != TW \
                        else tbl[CB[c]:CB[c] + CROWS[c], :]
                    if elem == TW:
                        nc.gpsimd.dma_gather(
                            gt[:], src_ap, ixt[:], NI, NI, TW,
                            single_packet=False,
                        )
                    else:
                        _dma_gather_raw(nc.gpsimd, gt[:], src_ap, ixt[:], NI, elem, TW)
                    for j in range(ncols):
                        t, q, st, sp_, tc_last = meta[c][g0 - sum(ncols_ch[:c]) + j]
                        if t not in pend:
                            pend[t] = ps.tile(
                                [P, width], dt.float32, tag="agg", bufs=2,
                                name=f"agg_{t}",
                            )
                        nc.tensor.matmul(
                            out=pend[t][SEG * q:SEG * (q + 1), :],
                            lhsT=smt[:, j, :], rhs=gt[:, j, :width],
                            start=st, stop=sp_,
                            tile_position=(0, SEG * q),
                        )
                        if tc_last:
                            pt = pend.pop(t)
                            if t not in first:
                                first[t] = 1
                                nc.vector.tensor_copy(out=acc[:, t, :], in_=pt[:])
                            else:
                                first[t] += 1
                                nc.vector.tensor_tensor(
                                    out=acc[:, t, :], in0=acc[:, t, :], in1=pt[:],
                                    op=mybir.AluOpType.add,
                                )
                            if first[t] == tile_nch[t]:
                                post_tile(t)
                for t in range(NW):
                    if tile_nch[t] == 0:
                        nc.vector.memset(acc[:, t, :], 0.0)
                        post_tile(t)

            # --- phase 2: L1 aggregate + elu' + t2 rows + chunked AllGather#2
            t2bufs = {}
            t2done = {}

            def make_t2(t):
                c = int(np.searchsorted(cfg["TSTART"], t, side="right") - 1)
                t0, t1 = int(cfg["TSTART"][c]), int(cfg["TSTART"][c + 1])
                if c not in t2bufs:
                    t2bufs[c] = hb.tile(
                        [P, t1 - t0, TW], dt.bfloat16, tag="t2b",
                        name=f"t2b_{c}",
                    )
                    t2done[c] = 0
                    nc.vector.memset(t2bufs[c][:].rearrange("p a b -> p (a b)"), 0.0)
                red = acc1[:, t, :]
                m = sp.tile([P, H], dt.float32, tag="m")
                nc.vector.tensor_scalar_min(out=m[:], in0=red, scalar1=0.0)
                e = sp.tile([P, H], dt.float32, tag="e")
                nc.scalar.activation(e[:], m[:], mybir.ActivationFunctionType.Exp)
                gpr = sp.tile([P, H], dt.bfloat16, tag="gpr")
                nc.vector.scalar_tensor_tensor(
                    out=gpr[:], in0=red, scalar=0.0, in1=e[:],
                    op0=mybir.AluOpType.max, op1=mybir.AluOpType.add,
                )
                tr = ps.tile([H, P], dt.bfloat16, tag="tr", bufs=2)
                nc.tensor.transpose(out=tr[:], in_=gpr[:], identity=ident[:])
                trsb = sp.tile([H, P], dt.bfloat16, tag="trsb")
                nc.vector.tensor_copy(out=trsb[:], in_=tr[:])
                t2p = ps.tile([P, C], dt.float32, tag="t2p", bufs=2)
                nc.tensor.matmul(out=t2p[:], lhsT=trsb[:], rhs=w2sb[:, :C],
                                 start=True, stop=True)
                nc.vector.tensor_tensor(
                    out=t2bufs[c][:, t - t0, :C], in0=t2p[:], in1=b2sb[:, :C],
                    op=mybir.AluOpType.add,
                )
                t2done[c] += 1
                if t2done[c] == t1 - t0:
                    nc.sync.dma_start(
                        out=t2k[CS[c]:CS[c] + (t1 - t0) * P, :].rearrange(
                            "(a p) w -> p a w", p=P),
                        in_=t2bufs[c][:],
                    )
                    nc.gpsimd.collective_compute(
                        "AllGather", mybir.AluOpType.bypass, replica_groups=rg,
                        ins=[t2k[CS[c]:CS[c] + CZ[c], :]],
                        outs=[tb2[CB[c] + 1:CB[c] + 1 + N_CORES * CZ[c], :]],
                    )

            agg_pass(tb1, elem1, H, acc1, make_t2)

            # --- phase 3: L2 aggregate + log_softmax
            def softmax_a(t):
                red = acc2[:, t, :]
                nm = sp.tile([P, 1], dt.float32, tag="nm")
                nc.vector.tensor_reduce(
                    out=nm[:], in_=red, axis=mybir.AxisListType.X,
                    op=mybir.AluOpType.max, negate=True,
                )
                sc = sp.tile([P, C], dt.float32, tag="sc")
                nc.scalar.activation(
                    sc[:], red, mybir.ActivationFunctionType.Exp,
                    bias=nm[:], accum_out=ssum[:, t:t + 1],
                )
                nc.vector.tensor_scalar(
                    out=outb[:, t, :], in0=red, scalar1=nm[:], scalar2=None,
                    op0=mybir.AluOpType.add,
                )

            agg_pass(tb2, elem2, C, acc2, softmax_a)

            nc.scalar.activation(lsm[:], ssum[:], mybir.ActivationFunctionType.Ln)
            for t in range(NW):
                nc.vector.tensor_scalar(
                    out=outb[:, t, :], in0=outb[:, t, :], scalar1=lsm[:, t:t + 1],
                    scalar2=None, op0=mybir.AluOpType.subtract,
                )
            nc.sync.dma_start(
                out=outp[:].rearrange("(t p) c -> p t c", p=P), in_=outb[:],
            )

    nc.compile()
    return nc


# ---------------------------------------------------------------- entry point

LAST_RESULT = {}


def _run(cfg, x, edge_index, W1, b1, W2, b2, trace=False):
    from concourse.bass_utils import run_bass_kernel_spmd

    sched, in_maps = host_prep(cfg, x, edge_index, W1, b1, W2, b2)
    # pad per-core idxp/smat to the common width
    totcols = sum(len(m) for m in sched["common_meta"])
    for im in in_maps:
        w = max(totcols, 1) * 8
        cur = im["idxp"].shape[1]
        assert cur == w, (cur, w)
        ws = max(totcols, 1) * SEG
        assert im["smat"].shape[1] == ws, (im["smat"].shape, ws)
    nc = build_program(cfg, sched)
    res = run_bass_kernel_spmd(
        nc, in_maps, list(range(N_CORES)), trace=trace,
    )
    LAST_RESULT["exec_time_ns"] = res.exec_time_ns
    LAST_RESULT["mean_exec_time_ns"] = res.mean_exec_time_ns
    N, NP, C = cfg["N"], cfg["NP"], cfg["C"]
    full = np.empty((N, C), dtype=np.float32)
    for k in range(N_CORES):
        outk = np.asarray(res.results[k]["out"], dtype=np.float32)
        full[k * NP:(k + 1) * NP] = outk[:NP]
    return full


def kernel(x, edge_index, W1, b1, W2, b2):
    trace = bool(int(os.environ.get("GNN_TRACE", "0")))
    return _run(FULL_CFG, x, edge_index, W1, b1, W2, b2, trace=trace)


# revision 15
# speedup vs baseline: 1.4722x; 1.4722x over previous
"""GNN message-passing (2-layer conv + log_softmax) as a Bass/Tile SPMD kernel
on 8 Trainium2 NeuronCores.

Strategy (dst-sharded 1D graph partition, replicated message tables):
  - nodes sharded 8-way; core k owns dst nodes [k*NP, (k+1)*NP)
  - L1: h1 = x@W1 + b1 computed on node shards (host-pretransposed bf16 xT),
    chunk-wise AllGather -> full bf16 table tb1 (chunk-major row layout)
  - aggregation: per-core dsts sorted by in-degree, grouped into 128-dst
    windows padded to the window max degree; messages fetched with indirect
    DMA gathers (row per edge slot, pad slots hit a zero row) and reduced on
    the TensorEngine by identity-matmul PSUM accumulation (exact fp32)
  - elu folded as g' = relu(f) + exp(min(f,0)) = elu(f)+1, compensated by
    passing b2' = b2 - W2.sum(0); t2 = g'@W2 + b2' built per window (PE
    transpose + matmul), AllGather#2 -> table tb2; second gather+reduce;
    log_softmax fused on ACT/DVE. Output rows are in per-core degree-perm
    order; the host inverts the permutation.
"""

import os
import sys

sys.path.insert(0, "/opt/trn_rl_repo")

import numpy as np
import ml_dtypes

BF16 = ml_dtypes.bfloat16

# static problem config (full-size); tests may build their own cfg
N_CORES = 8
P = 128


def _make_cfg(n_nodes, n_edges, f_in=512, hid=64, n_cls=40, ctarget=256):
    np_ = n_nodes // N_CORES
    assert np_ * N_CORES == n_nodes
    nw = (np_ + P - 1) // P
    npad = nw * P
    n_chunks = min(4, nw)
    # chunk boundaries in units of 128-row tiles
    tiles = [nw // n_chunks + (1 if i < nw % n_chunks else 0) for i in range(n_chunks)]
    tstart = np.concatenate([[0], np.cumsum(tiles)])
    # table1 chunks cover real local rows [t0*128, min(t1*128, np_))
    c1_start = [int(min(tstart[i] * P, np_)) for i in range(n_chunks + 1)]
    c1_size = [c1_start[i + 1] - c1_start[i] for i in range(n_chunks)]
    # table2 chunks cover padded rows [t0*128, t1*128)
    c2_start = [int(tstart[i] * P) for i in range(n_chunks + 1)]
    c2_size = [c2_start[i + 1] - c2_start[i] for i in range(n_chunks)]
    base1 = np.concatenate([[0], np.cumsum([N_CORES * s for s in c1_size])])
    base2 = np.concatenate([[0], np.cumsum([N_CORES * s for s in c2_size])])
    tot1 = int(base1[-1])  # == n_nodes
    tot2 = int(base2[-1])  # == 8 * npad
    return dict(
        N=n_nodes, E=n_edges, F=f_in, H=hid, C=n_cls, NP=np_, NW=nw, NPAD=npad,
        NCH=n_chunks, TILES=tiles, TSTART=tstart,
        C1S=c1_start, C1Z=c1_size, C2S=c2_start, C2Z=c2_size,
        BASE1=base1, BASE2=base2, TOT1=tot1, TOT2=tot2,
        ZROW1=tot1, ZROW2=tot2, CTARGET=ctarget,
    )


FULL_CFG = _make_cfg(100000, 3200000)


# ---------------------------------------------------------------- host prep

def _row_maps(cfg, pos_all):
    """map global node id -> table1 row / table2 row (chunk-major layouts).
    pos_all: [N] position of each node within its core's degree-perm."""
    N, NP = cfg["N"], cfg["NP"]
    g = np.arange(N, dtype=np.int64)
    r = g // NP
    l = g % NP
    c1b = np.asarray(cfg["C1S"])
    c = np.searchsorted(c1b, l, side="right") - 1
    sz = np.asarray(cfg["C1Z"] + [1])[c]
    map1 = np.asarray(cfg["BASE1"])[c] + r * sz + (l - c1b[c])
    p = pos_all
    c2b = np.asarray(cfg["C2S"])
    c2 = np.searchsorted(c2b, p, side="right") - 1
    sz2 = np.asarray(cfg["C2Z"] + [1])[c2]
    map2 = np.asarray(cfg["BASE2"])[c2] + r * sz2 + (p - c2b[c2])
    map1 = np.concatenate([map1, [cfg["ZROW1"]]]).astype(np.int32)
    map2 = np.concatenate([map2, [cfg["ZROW2"]]]).astype(np.int32)
    return map1, map2


def host_prep(cfg, x, edge_index, W1, b1, W2, b2):
    N, NP, NW = cfg["N"], cfg["NP"], cfg["NW"]
    src = np.asarray(edge_index[0]).astype(np.int64)
    dst = np.asarray(edge_index[1]).astype(np.int64)

    per_core = []
    for k in range(N_CORES):
        sel = (dst >= k * NP) & (dst < (k + 1) * NP)
        s_k = src[sel]
        d_k = (dst[sel] - k * NP).astype(np.int64)
        deg = np.bincount(d_k, minlength=NP)
        perm = np.argsort(-deg, kind="stable")
        pos = np.empty(NP, dtype=np.int64)
        pos[perm] = np.arange(NP)
        order = np.argsort(d_k, kind="stable")
        ss = s_k[order]
        starts = np.concatenate([[0], np.cumsum(deg)])
        per_core.append(dict(deg=deg, perm=perm, pos=pos, ss=ss, starts=starts))

    # window capacities (uniform across cores)
    D = np.zeros(NW, dtype=np.int64)
    for k in range(N_CORES):
        deg, perm = per_core[k]["deg"], per_core[k]["perm"]
        for w in range(NW):
            n0 = perm[w * P] if w * P < NP else None
            dw = int(deg[n0]) if n0 is not None else 0
            D[w] = max(D[w], dw)
    D = np.maximum(D, 1)

    # greedy grouping of windows into gather calls
    groups = []  # (list of w, list of D_w, colstart)
    cur, curD = [], 0
    for w in range(NW):
        if cur and curD + D[w] > cfg["CTARGET"]:
            groups.append((cur, curD))
            cur, curD = [], 0
        cur.append(w)
        curD += int(D[w])
    if cur:
        groups.append((cur, curD))
    woff = np.concatenate([[0], np.cumsum(D)])  # col offset per window
    sumc = int(woff[-1])

    # raw src blocks per core (sentinel N for padding), then remap
    pos_all = np.concatenate([pc["pos"] for pc in per_core])
    map1, map2 = _row_maps(cfg, pos_all)
    idx1, idx2 = [], []
    for k in range(N_CORES):
        pc = per_core[k]
        raw = np.full((P, sumc), N, dtype=np.int64)
        deg, perm, ss, starts = pc["deg"], pc["perm"], pc["ss"], pc["starts"]
        for w in range(NW):
            for p in range(min(P, NP - w * P)):
                n = perm[w * P + p]
                dn = deg[n]
                if dn:
                    raw[p, woff[w]:woff[w] + dn] = ss[starts[n]:starts[n] + dn]
        idx1.append(map1[raw])
        idx2.append(map2[raw])

    # per-core tensors
    W1b = np.asarray(W1, dtype=np.float32).astype(BF16)
    W2b = np.asarray(W2, dtype=np.float32).astype(BF16)
    b1r = np.tile(np.asarray(b1, dtype=np.float32)[None, :], (P, 1))
    b2a = np.asarray(b2, dtype=np.float32) - np.asarray(W2, np.float32).sum(0)
    b2r = np.tile(b2a[None, :], (P, 1))
    in_maps = []
    xf = np.asarray(x, dtype=np.float32)
    for k in range(N_CORES):
        xT = np.ascontiguousarray(xf[k * NP:(k + 1) * NP].T).astype(BF16)
        in_maps.append(dict(
            xT=xT, W1=W1b, b1r=b1r, W2=W2b, b2r=b2r,
            idx1=idx1[k], idx2=idx2[k],
        ))
    sched = dict(D=D, groups=groups, woff=woff, sumc=sumc)
    perms = [pc["perm"] for pc in per_core]
    return sched, in_maps, perms


# ---------------------------------------------------------------- device code

def build_program(cfg, sched):
    import concourse.bass as bass
    import concourse.bacc as bacc
    import concourse.mybir as mybir
    from concourse.tile import TileContext
    from concourse.masks import make_identity

    dt = mybir.dt
    N, F, H, C = cfg["N"], cfg["F"], cfg["H"], cfg["C"]
    NP, NW, NPAD, NCH = cfg["NP"], cfg["NW"], cfg["NPAD"], cfg["NCH"]
    D, groups, woff, sumc = sched["D"], sched["groups"], sched["woff"], sched["sumc"]
    KF = F // P

    nc = bacc.Bacc(
        "TRN2", target_bir_lowering=False, debug=False, num_devices=N_CORES
    )
    xT = nc.declare_dram_parameter("xT", [F, NP], dt.bfloat16, isOutput=False)
    W1p = nc.declare_dram_parameter("W1", [F, H], dt.bfloat16, isOutput=False)
    b1p = nc.declare_dram_parameter("b1r", [P, H], dt.float32, isOutput=False)
    W2p = nc.declare_dram_parameter("W2", [H, C], dt.bfloat16, isOutput=False)
    b2p = nc.declare_dram_parameter("b2r", [P, C], dt.float32, isOutput=False)
    ix1p = nc.declare_dram_parameter("idx1", [P, sumc], dt.int32, isOutput=False)
    ix2p = nc.declare_dram_parameter("idx2", [P, sumc], dt.int32, isOutput=False)
    outp = nc.declare_dram_parameter("out", [NPAD, C], dt.float32, isOutput=True)

    rg = [list(range(N_CORES))]
    cmax = max(cD for _, cD in groups)

    with TileContext(nc) as tc:
        with (
            tc.tile_pool(name="const", bufs=1) as const,
            tc.tile_pool(name="dram", bufs=1, space="DRAM") as dram,
            tc.tile_pool(name="xp", bufs=3) as xp,
            tc.tile_pool(name="hp", bufs=3) as hp,
            tc.tile_pool(name="ixp", bufs=1) as ixp,
            tc.tile_pool(name="gp", bufs=96) as gpl,
            tc.tile_pool(name="sp", bufs=3) as sp,
            tc.tile_pool(name="ps", bufs=2, space="PSUM") as ps,
        ):
            # --- constants
            w1sb = const.tile([P, KF, H], dt.bfloat16)
            nc.sync.dma_start(out=w1sb[:], in_=W1p[:].rearrange("(c p) h -> p c h", p=P))
            w2sb = const.tile([H, C], dt.bfloat16)
            nc.sync.dma_start(out=w2sb[:], in_=W2p[:])
            b1sb = const.tile([P, H], dt.float32)
            nc.sync.dma_start(out=b1sb[:], in_=b1p[:])
            b2sb = const.tile([P, C], dt.float32)
            nc.sync.dma_start(out=b2sb[:], in_=b2p[:])
            ident = const.tile([P, P], dt.bfloat16)
            make_identity(nc, ident[:])

            # --- internal DRAM
            h1k = dram.tile([NP, H], dt.bfloat16)
            t2k = dram.tile([NPAD, C], dt.bfloat16)
            tb1 = dram.tile([cfg["TOT1"] + 1, H], dt.bfloat16)
            tb2 = dram.tile([cfg["TOT2"] + 1, C], dt.bfloat16)

            # zero rows for padding slots
            zt = const.tile([1, H], dt.bfloat16)
            nc.gpsimd.memset(zt[:], 0.0)
            nc.sync.dma_start(out=tb1[cfg["ZROW1"]:cfg["ZROW1"] + 1, :], in_=zt[:, :H])
            nc.sync.dma_start(out=tb2[cfg["ZROW2"]:cfg["ZROW2"] + 1, :], in_=zt[:, :C])

            # --- phase 1: h1 = x@W1 + b1 on local shard, chunked AllGather
            xTr = xT[:].rearrange("(c p) n -> p c n", p=P)
            for ch in range(NCH):
                t0, t1 = int(cfg["TSTART"][ch]), int(cfg["TSTART"][ch + 1])
                for nt in range(t0, t1):
                    cs = min(P, NP - nt * P)
                    if cs <= 0:
                        continue
                    xt = xp.tile([P, KF, P], dt.bfloat16, tag="xt")
                    nc.sync.dma_start(out=xt[:, :, :cs], in_=xTr[:, :, nt * P:nt * P + cs])
                    ph = ps.tile([P, H], dt.float32, tag="ph")
                    for kf in range(KF):
                        nc.tensor.matmul(
                            out=ph[:cs, :], lhsT=xt[:, kf, :cs], rhs=w1sb[:, kf, :],
                            start=(kf == 0), stop=(kf == KF - 1),
                        )
                    h1sb = hp.tile([P, H], dt.bfloat16, tag="h1sb")
                    nc.vector.tensor_tensor(
                        out=h1sb[:cs, :], in0=ph[:cs, :], in1=b1sb[:cs, :],
                        op=mybir.AluOpType.add,
                    )
                    nc.sync.dma_start(out=h1k[nt * P:nt * P + cs, :], in_=h1sb[:cs, :])
                # gather this chunk of h1 across cores
                s0, sz = cfg["C1S"][ch], cfg["C1Z"][ch]
                nc.gpsimd.collective_compute(
                    "AllGather", mybir.AluOpType.bypass, replica_groups=rg,
                    ins=[h1k[s0:s0 + sz, :]],
                    outs=[tb1[int(cfg["BASE1"][ch]):int(cfg["BASE1"][ch]) + N_CORES * sz, :]],
                )

            # --- phase 2: L1 gather+reduce, elu', t2 rows, chunked AllGather#2
            ch_end = {int(cfg["TSTART"][ch + 1]) - 1: ch for ch in range(NCH)}
            ixsb1 = ixp.tile([P, sumc], dt.int32, tag="ixsb1")
            nc.sync.dma_start(out=ixsb1[:], in_=ix1p[:])
            for w in range(NW):
                dw = int(D[w])
                c0 = int(woff[w])
                gts = []
                for s in range(dw):
                    gt = gpl.tile([P, H], dt.bfloat16, tag="gt")
                    nc.gpsimd.indirect_dma_start(
                        out=gt[:], out_offset=None,
                        in_=tb1[:],
                        in_offset=bass.IndirectOffsetOnAxis(
                            ap=ixsb1[:, c0 + s:c0 + s + 1], axis=0),
                    )
                    gts.append(gt)
                red = ps.tile([P, H], dt.float32, tag="red")
                for s in range(dw):
                    nc.tensor.matmul(
                        out=red[:, :], lhsT=ident[:],
                        rhs=gts[s][:],
                        start=(s == 0), stop=(s == dw - 1),
                    )
                if True:
                    # g' = relu(f) + exp(min(f, 0))  (= elu(f) + 1)
                    m = sp.tile([P, H], dt.float32, tag="m")
                    nc.vector.tensor_scalar_min(out=m[:], in0=red[:], scalar1=0.0)
                    e = sp.tile([P, H], dt.float32, tag="e")
                    nc.scalar.activation(e[:], m[:], mybir.ActivationFunctionType.Exp)
                    gpr = sp.tile([P, H], dt.bfloat16, tag="gpr")
                    nc.vector.scalar_tensor_tensor(
                        out=gpr[:], in0=red[:], scalar=0.0, in1=e[:],
                        op0=mybir.AluOpType.max, op1=mybir.AluOpType.add,
                    )
                    # t2 row block = g'@W2 + b2'
                    tr = ps.tile([H, P], dt.bfloat16, tag="tr")
                    nc.tensor.transpose(out=tr[:], in_=gpr[:], identity=ident[:])
                    trsb = sp.tile([H, P], dt.bfloat16, tag="trsb")
                    nc.vector.tensor_copy(out=trsb[:], in_=tr[:])
                    t2p = ps.tile([P, C], dt.float32, tag="t2p")
                    nc.tensor.matmul(out=t2p[:], lhsT=trsb[:], rhs=w2sb[:],
                                     start=True, stop=True)
                    t2sb = sp.tile([P, C], dt.bfloat16, tag="t2sb")
                    nc.vector.tensor_tensor(out=t2sb[:], in0=t2p[:], in1=b2sb[:, :C],
                                            op=mybir.AluOpType.add)
                    nc.sync.dma_start(out=t2k[w * P:(w + 1) * P, :], in_=t2sb[:])
                    if w in ch_end:
                        ch = ch_end[w]
                        s0, sz = cfg["C2S"][ch], cfg["C2Z"][ch]
                        nc.gpsimd.collective_compute(
                            "AllGather", mybir.AluOpType.bypass, replica_groups=rg,
                            ins=[t2k[s0:s0 + sz, :]],
                            outs=[tb2[int(cfg["BASE2"][ch]):int(cfg["BASE2"][ch]) + N_CORES * sz, :]],
                        )

            # --- phase 3: L2 gather+reduce + log_softmax
            ixsb2 = ixp.tile([P, sumc], dt.int32, tag="ixsb2")
            nc.sync.dma_start(out=ixsb2[:], in_=ix2p[:])
            for w in range(NW):
                dw = int(D[w])
                c0 = int(woff[w])
                gts = []
                for s in range(dw):
                    gt = gpl.tile([P, C], dt.bfloat16, tag="gt2")
                    nc.gpsimd.indirect_dma_start(
                        out=gt[:], out_offset=None,
                        in_=tb2[:],
                        in_offset=bass.IndirectOffsetOnAxis(
                            ap=ixsb2[:, c0 + s:c0 + s + 1], axis=0),
                    )
                    gts.append(gt)
                red = ps.tile([P, C], dt.float32, tag="red")
                for s in range(dw):
                    nc.tensor.matmul(
                        out=red[:, :], lhsT=ident[:],
                        rhs=gts[s][:],
                        start=(s == 0), stop=(s == dw - 1),
                    )
                if True:
                    # log_softmax over classes
                    nm = sp.tile([P, 1], dt.float32, tag="nm")
                    nc.vector.tensor_reduce(
                        out=nm[:], in_=red[:], axis=mybir.AxisListType.X,
                        op=mybir.AluOpType.max, negate=True,
                    )
                    sc = sp.tile([P, C], dt.float32, tag="sc")
                    ssum = sp.tile([P, 1], dt.float32, tag="ssum")
                    nc.scalar.activation(
                        sc[:], red[:], mybir.ActivationFunctionType.Exp,
                        bias=nm[:], accum_out=ssum[:],
                    )
                    ls = sp.tile([P, 1], dt.float32, tag="ls")
                    nc.scalar.activation(ls[:], ssum[:], mybir.ActivationFunctionType.Ln)
                    ob = sp.tile([P, C], dt.float32, tag="ob")
                    nc.vector.tensor_scalar(
                        out=ob[:], in0=red[:], scalar1=nm[:], scalar2=ls[:],
                        op0=mybir.AluOpType.add, op1=mybir.AluOpType.subtract,
                    )
                    nc.sync.dma_start(out=outp[w * P:(w + 1) * P, :], in_=ob[:])

    nc.compile()
    return nc


# ---------------------------------------------------------------- entry point

LAST_RESULT = {}


def _run(cfg, x, edge_index, W1, b1, W2, b2, trace=False):
    from concourse.bass_utils import run_bass_kernel_spmd

    sched, in_maps, perms = host_prep(cfg, x, edge_index, W1, b1, W2, b2)
    nc = build_program(cfg, sched)
    res = run_bass_kernel_spmd(
        nc, in_maps, list(range(N_CORES)), trace=trace,
    )
    LAST_RESULT["exec_time_ns"] = res.exec_time_ns
    LAST_RESULT["mean_exec_time_ns"] = res.mean_exec_time_ns
    N, NP, C = cfg["N"], cfg["NP"], cfg["C"]
    full = np.empty((N, C), dtype=np.float32)
    for k in range(N_CORES):
        outk = np.asarray(res.results[k]["out"], dtype=np.float32)
        blk = full[k * NP:(k + 1) * NP]
        blk[perms[k]] = outk[:NP]
    return full


def kernel(x, edge_index, W1, b1, W2, b2):
    trace = bool(int(os.environ.get("GNN_TRACE", "0")))
    return _run(FULL_CFG, x, edge_index, W1, b1, W2, b2, trace=trace)



# revision 16
# speedup vs baseline: 1.7295x; 1.1748x over previous
"""GNN message-passing (2-layer conv + log_softmax) as a Bass/Tile SPMD kernel
on 8 Trainium2 NeuronCores.

Strategy (dst-sharded 1D graph partition, replicated message tables):
  - nodes sharded 8-way; core k owns dst nodes [k*NP, (k+1)*NP)
  - L1: h1 = x@W1 + b1 computed on node shards (host-pretransposed bf16 xT),
    chunk-wise AllGather -> full bf16 table tb1 (chunk-major row layout)
  - aggregation: per-core dsts sorted by in-degree, grouped into 128-dst
    windows padded to the window max degree; messages fetched with indirect
    DMA gathers (row per edge slot, pad slots hit a zero row) and reduced on
    the TensorEngine by identity-matmul PSUM accumulation (exact fp32)
  - elu folded as g' = relu(f) + exp(min(f,0)) = elu(f)+1, compensated by
    passing b2' = b2 - W2.sum(0); t2 = g'@W2 + b2' built per window (PE
    transpose + matmul), AllGather#2 -> table tb2; second gather+reduce;
    log_softmax fused on ACT/DVE. Output rows are in per-core degree-perm
    order; the host inverts the permutation.
"""

import os
import sys

sys.path.insert(0, "/opt/trn_rl_repo")

import numpy as np
import ml_dtypes

BF16 = ml_dtypes.bfloat16

# static problem config (full-size); tests may build their own cfg
N_CORES = 8
P = 128


def _make_cfg(n_nodes, n_edges, f_in=512, hid=64, n_cls=40, ctarget=256):
    np_ = n_nodes // N_CORES
    assert np_ * N_CORES == n_nodes
    nw = (np_ + P - 1) // P
    npad = nw * P
    n_chunks = min(4, nw)
    # chunk boundaries in units of 128-row tiles
    tiles = [nw // n_chunks + (1 if i < nw % n_chunks else 0) for i in range(n_chunks)]
    tstart = np.concatenate([[0], np.cumsum(tiles)])
    # table1 chunks cover real local rows [t0*128, min(t1*128, np_))
    c1_start = [int(min(tstart[i] * P, np_)) for i in range(n_chunks + 1)]
    c1_size = [c1_start[i + 1] - c1_start[i] for i in range(n_chunks)]
    # table2 chunks cover padded rows [t0*128, t1*128)
    c2_start = [int(tstart[i] * P) for i in range(n_chunks + 1)]
    c2_size = [c2_start[i + 1] - c2_start[i] for i in range(n_chunks)]
    base1 = np.concatenate([[0], np.cumsum([N_CORES * s for s in c1_size])])
    base2 = np.concatenate([[0], np.cumsum([N_CORES * s for s in c2_size])])
    tot1 = int(base1[-1])  # == n_nodes
    tot2 = int(base2[-1])  # == 8 * npad
    return dict(
        N=n_nodes, E=n_edges, F=f_in, H=hid, C=n_cls, NP=np_, NW=nw, NPAD=npad,
        NCH=n_chunks, TILES=tiles, TSTART=tstart,
        C1S=c1_start, C1Z=c1_size, C2S=c2_start, C2Z=c2_size,
        BASE1=base1, BASE2=base2, TOT1=tot1, TOT2=tot2,
        ZROW1=tot1, ZROW2=tot2, CTARGET=ctarget,
    )


FULL_CFG = _make_cfg(100000, 3200000)


# ---------------------------------------------------------------- host prep

def _row_maps(cfg, pos_all):
    """map global node id -> table1 row / table2 row (chunk-major layouts).
    pos_all: [N] position of each node within its core's degree-perm."""
    N, NP = cfg["N"], cfg["NP"]
    g = np.arange(N, dtype=np.int64)
    r = g // NP
    l = g % NP
    c1b = np.asarray(cfg["C1S"])
    c = np.searchsorted(c1b, l, side="right") - 1
    sz = np.asarray(cfg["C1Z"] + [1])[c]
    map1 = np.asarray(cfg["BASE1"])[c] + r * sz + (l - c1b[c])
    p = pos_all
    c2b = np.asarray(cfg["C2S"])
    c2 = np.searchsorted(c2b, p, side="right") - 1
    sz2 = np.asarray(cfg["C2Z"] + [1])[c2]
    map2 = np.asarray(cfg["BASE2"])[c2] + r * sz2 + (p - c2b[c2])
    map1 = np.concatenate([map1, [cfg["ZROW1"]]]).astype(np.int32)
    map2 = np.concatenate([map2, [cfg["ZROW2"]]]).astype(np.int32)
    return map1, map2


def host_prep(cfg, x, edge_index, W1, b1, W2, b2):
    N, NP, NW = cfg["N"], cfg["NP"], cfg["NW"]
    src = np.asarray(edge_index[0]).astype(np.int64)
    dst = np.asarray(edge_index[1]).astype(np.int64)

    per_core = []
    for k in range(N_CORES):
        sel = (dst >= k * NP) & (dst < (k + 1) * NP)
        s_k = src[sel]
        d_k = (dst[sel] - k * NP).astype(np.int64)
        deg = np.bincount(d_k, minlength=NP)
        perm = np.argsort(-deg, kind="stable")
        pos = np.empty(NP, dtype=np.int64)
        pos[perm] = np.arange(NP)
        order = np.argsort(d_k, kind="stable")
        ss = s_k[order]
        starts = np.concatenate([[0], np.cumsum(deg)])
        per_core.append(dict(deg=deg, perm=perm, pos=pos, ss=ss, starts=starts))

    # window capacities (uniform across cores)
    D = np.zeros(NW, dtype=np.int64)
    for k in range(N_CORES):
        deg, perm = per_core[k]["deg"], per_core[k]["perm"]
        for w in range(NW):
            n0 = perm[w * P] if w * P < NP else None
            dw = int(deg[n0]) if n0 is not None else 0
            D[w] = max(D[w], dw)
    D = np.maximum(D, 1)

    # greedy grouping of windows into gather calls
    groups = []  # (list of w, list of D_w, colstart)
    cur, curD = [], 0
    for w in range(NW):
        if cur and curD + D[w] > cfg["CTARGET"]:
            groups.append((cur, curD))
            cur, curD = [], 0
        cur.append(w)
        curD += int(D[w])
    if cur:
        groups.append((cur, curD))
    woff = np.concatenate([[0], np.cumsum(D)])  # col offset per window
    sumc = int(woff[-1])

    # raw src blocks per core (sentinel N for padding), then remap
    pos_all = np.concatenate([pc["pos"] for pc in per_core])
    map1, map2 = _row_maps(cfg, pos_all)
    idx1, idx2 = [], []
    for k in range(N_CORES):
        pc = per_core[k]
        raw = np.full((P, sumc), N, dtype=np.int64)
        deg, perm, ss, starts = pc["deg"], pc["perm"], pc["ss"], pc["starts"]
        for w in range(NW):
            for p in range(min(P, NP - w * P)):
                n = perm[w * P + p]
                dn = deg[n]
                if dn:
                    raw[p, woff[w]:woff[w] + dn] = ss[starts[n]:starts[n] + dn]
        idx1.append(map1[raw])
        idx2.append(map2[raw])

    # per-core tensors
    W1b = np.asarray(W1, dtype=np.float32).astype(BF16)
    W2b = np.asarray(W2, dtype=np.float32).astype(BF16)
    b1r = np.tile(np.asarray(b1, dtype=np.float32)[None, :], (P, 1))
    b2a = np.asarray(b2, dtype=np.float32) - np.asarray(W2, np.float32).sum(0)
    b2r = np.tile(b2a[None, :], (P, 1))
    in_maps = []
    xf = np.asarray(x, dtype=np.float32)
    for k in range(N_CORES):
        xT = np.ascontiguousarray(xf[k * NP:(k + 1) * NP].T).astype(BF16)
        in_maps.append(dict(
            xT=xT, W1=W1b, b1r=b1r, W2=W2b, b2r=b2r,
            idx1=idx1[k], idx2=idx2[k],
        ))
    sched = dict(D=D, groups=groups, woff=woff, sumc=sumc)
    perms = [pc["perm"] for pc in per_core]
    return sched, in_maps, perms


# ---------------------------------------------------------------- device code

def build_program(cfg, sched):
    import concourse.bass as bass
    import concourse.bacc as bacc
    import concourse.mybir as mybir
    from concourse.tile import TileContext
    from concourse.masks import make_identity

    dt = mybir.dt
    N, F, H, C = cfg["N"], cfg["F"], cfg["H"], cfg["C"]
    NP, NW, NPAD, NCH = cfg["NP"], cfg["NW"], cfg["NPAD"], cfg["NCH"]
    D, groups, woff, sumc = sched["D"], sched["groups"], sched["woff"], sched["sumc"]
    KF = F // P

    nc = bacc.Bacc(
        "TRN2", target_bir_lowering=False, debug=False, num_devices=N_CORES
    )
    xT = nc.declare_dram_parameter("xT", [F, NP], dt.bfloat16, isOutput=False)
    W1p = nc.declare_dram_parameter("W1", [F, H], dt.bfloat16, isOutput=False)
    b1p = nc.declare_dram_parameter("b1r", [P, H], dt.float32, isOutput=False)
    W2p = nc.declare_dram_parameter("W2", [H, C], dt.bfloat16, isOutput=False)
    b2p = nc.declare_dram_parameter("b2r", [P, C], dt.float32, isOutput=False)
    ix1p = nc.declare_dram_parameter("idx1", [P, sumc], dt.int32, isOutput=False)
    ix2p = nc.declare_dram_parameter("idx2", [P, sumc], dt.int32, isOutput=False)
    outp = nc.declare_dram_parameter("out", [NPAD, C], dt.float32, isOutput=True)

    rg = [list(range(N_CORES))]
    cmax = max(cD for _, cD in groups)

    with TileContext(nc) as tc:
        with (
            tc.tile_pool(name="const", bufs=1) as const,
            tc.tile_pool(name="dram", bufs=1, space="DRAM") as dram,
            tc.tile_pool(name="xp", bufs=3) as xp,
            tc.tile_pool(name="hp", bufs=3) as hp,
            tc.tile_pool(name="ixp", bufs=8) as ixp,
            tc.tile_pool(name="gp", bufs=8) as gpl,
            tc.tile_pool(name="sp", bufs=3) as sp,
            tc.tile_pool(name="ps", bufs=2, space="PSUM") as ps,
        ):
            # --- constants
            w1sb = const.tile([P, KF, H], dt.bfloat16)
            nc.sync.dma_start(out=w1sb[:], in_=W1p[:].rearrange("(c p) h -> p c h", p=P))
            w2sb = const.tile([H, C], dt.bfloat16)
            nc.sync.dma_start(out=w2sb[:], in_=W2p[:])
            b1sb = const.tile([P, H], dt.float32)
            nc.sync.dma_start(out=b1sb[:], in_=b1p[:])
            b2sb = const.tile([P, C], dt.float32)
            nc.sync.dma_start(out=b2sb[:], in_=b2p[:])
            ident = const.tile([P, P], dt.bfloat16)
            make_identity(nc, ident[:])

            # --- internal DRAM
            h1k = dram.tile([NP, H], dt.bfloat16)
            t2k = dram.tile([NPAD, C], dt.bfloat16)
            tb1 = dram.tile([cfg["TOT1"] + 1, H], dt.bfloat16)
            tb2 = dram.tile([cfg["TOT2"] + 1, C], dt.bfloat16)

            # zero rows for padding slots
            zt = const.tile([1, H], dt.bfloat16)
            nc.gpsimd.memset(zt[:], 0.0)
            nc.sync.dma_start(out=tb1[cfg["ZROW1"]:cfg["ZROW1"] + 1, :], in_=zt[:, :H])
            nc.sync.dma_start(out=tb2[cfg["ZROW2"]:cfg["ZROW2"] + 1, :], in_=zt[:, :C])

            # --- phase 1: h1 = x@W1 + b1 on local shard, chunked AllGather
            xTr = xT[:].rearrange("(c p) n -> p c n", p=P)
            for ch in range(NCH):
                t0, t1 = int(cfg["TSTART"][ch]), int(cfg["TSTART"][ch + 1])
                for nt in range(t0, t1):
                    cs = min(P, NP - nt * P)
                    if cs <= 0:
                        continue
                    xt = xp.tile([P, KF, P], dt.bfloat16, tag="xt")
                    nc.sync.dma_start(out=xt[:, :, :cs], in_=xTr[:, :, nt * P:nt * P + cs])
                    ph = ps.tile([P, H], dt.float32, tag="ph")
                    for kf in range(KF):
                        nc.tensor.matmul(
                            out=ph[:cs, :], lhsT=xt[:, kf, :cs], rhs=w1sb[:, kf, :],
                            start=(kf == 0), stop=(kf == KF - 1),
                        )
                    h1sb = hp.tile([P, H], dt.bfloat16, tag="h1sb")
                    nc.vector.tensor_tensor(
                        out=h1sb[:cs, :], in0=ph[:cs, :], in1=b1sb[:cs, :],
                        op=mybir.AluOpType.add,
                    )
                    nc.sync.dma_start(out=h1k[nt * P:nt * P + cs, :], in_=h1sb[:cs, :])
                # gather this chunk of h1 across cores
                s0, sz = cfg["C1S"][ch], cfg["C1Z"][ch]
                nc.gpsimd.collective_compute(
                    "AllGather", mybir.AluOpType.bypass, replica_groups=rg,
                    ins=[h1k[s0:s0 + sz, :]],
                    outs=[tb1[int(cfg["BASE1"][ch]):int(cfg["BASE1"][ch]) + N_CORES * sz, :]],
                )

            # --- phase 2: L1 gather+reduce, elu', t2 rows, chunked AllGather#2
            ch_end = {int(cfg["TSTART"][ch + 1]) - 1: ch for ch in range(NCH)}
            for w in range(NW):
                dw = int(D[w])
                c0 = int(woff[w])
                gts = []
                for s in range(dw):
                    ix = ixp.tile([P, 1], dt.int32, tag="ix")
                    nc.sync.dma_start(out=ix[:], in_=ix1p[:, c0 + s:c0 + s + 1])
                    gt = gpl.tile([P, H], dt.bfloat16, tag="gt")
                    nc.gpsimd.indirect_dma_start(
                        out=gt[:], out_offset=None,
                        in_=tb1[:],
                        in_offset=bass.IndirectOffsetOnAxis(ap=ix[:, :1], axis=0),
                    )
                    gts.append(gt)
                red = ps.tile([P, H], dt.float32, tag="red")
                for s in range(dw):
                    nc.tensor.matmul(
                        out=red[:, :], lhsT=ident[:],
                        rhs=gts[s][:],
                        start=(s == 0), stop=(s == dw - 1),
                    )
                if True:
                    # g' = relu(f) + exp(min(f, 0))  (= elu(f) + 1)
                    m = sp.tile([P, H], dt.float32, tag="m")
                    nc.vector.tensor_scalar_min(out=m[:], in0=red[:], scalar1=0.0)
                    e = sp.tile([P, H], dt.float32, tag="e")
                    nc.scalar.activation(e[:], m[:], mybir.ActivationFunctionType.Exp)
                    gpr = sp.tile([P, H], dt.bfloat16, tag="gpr")
                    nc.vector.scalar_tensor_tensor(
                        out=gpr[:], in0=red[:], scalar=0.0, in1=e[:],
                        op0=mybir.AluOpType.max, op1=mybir.AluOpType.add,
                    )
                    # t2 row block = g'@W2 + b2'
                    tr = ps.tile([H, P], dt.bfloat16, tag="tr")
                    nc.tensor.transpose(out=tr[:], in_=gpr[:], identity=ident[:])
                    trsb = sp.tile([H, P], dt.bfloat16, tag="trsb")
                    nc.vector.tensor_copy(out=trsb[:], in_=tr[:])
                    t2p = ps.tile([P, C], dt.float32, tag="t2p")
                    nc.tensor.matmul(out=t2p[:], lhsT=trsb[:], rhs=w2sb[:],
                                     start=True, stop=True)
                    t2sb = sp.tile([P, C], dt.bfloat16, tag="t2sb")
                    nc.vector.tensor_tensor(out=t2sb[:], in0=t2p[:], in1=b2sb[:, :C],
                                            op=mybir.AluOpType.add)
                    nc.sync.dma_start(out=t2k[w * P:(w + 1) * P, :], in_=t2sb[:])
                    if w in ch_end:
                        ch = ch_end[w]
                        s0, sz = cfg["C2S"][ch], cfg["C2Z"][ch]
                        nc.gpsimd.collective_compute(
                            "AllGather", mybir.AluOpType.bypass, replica_groups=rg,
                            ins=[t2k[s0:s0 + sz, :]],
                            outs=[tb2[int(cfg["BASE2"][ch]):int(cfg["BASE2"][ch]) + N_CORES * sz, :]],
                        )

            # --- phase 3: L2 gather+reduce + log_softmax
            for w in range(NW):
                dw = int(D[w])
                c0 = int(woff[w])
                gts = []
                for s in range(dw):
                    ix = ixp.tile([P, 1], dt.int32, tag="ix2")
                    nc.sync.dma_start(out=ix[:], in_=ix2p[:, c0 + s:c0 + s + 1])
                    gt = gpl.tile([P, C], dt.bfloat16, tag="gt2")
                    nc.gpsimd.indirect_dma_start(
                        out=gt[:], out_offset=None,
                        in_=tb2[:],
                        in_offset=bass.IndirectOffsetOnAxis(ap=ix[:, :1], axis=0),
                    )
                    gts.append(gt)
                red = ps.tile([P, C], dt.float32, tag="red")
                for s in range(dw):
                    nc.tensor.matmul(
                        out=red[:, :], lhsT=ident[:],
                        rhs=gts[s][:],
                        start=(s == 0), stop=(s == dw - 1),
                    )
                if True:
                    # log_softmax over classes
                    nm = sp.tile([P, 1], dt.float32, tag="nm")
                    nc.vector.tensor_reduce(
                        out=nm[:], in_=red[:], axis=mybir.AxisListType.X,
                        op=mybir.AluOpType.max, negate=True,
                    )
                    sc = sp.tile([P, C], dt.float32, tag="sc")
                    ssum = sp.tile([P, 1], dt.float32, tag="ssum")
                    nc.scalar.activation(
                        sc[:], red[:], mybir.ActivationFunctionType.Exp,
                        bias=nm[:], accum_out=ssum[:],
                    )
                    ls = sp.tile([P, 1], dt.float32, tag="ls")
                    nc.scalar.activation(ls[:], ssum[:], mybir.ActivationFunctionType.Ln)
                    ob = sp.tile([P, C], dt.float32, tag="ob")
                    nc.vector.tensor_scalar(
                        out=ob[:], in0=red[:], scalar1=nm[:], scalar2=ls[:],
                        op0=mybir.AluOpType.add, op1=mybir.AluOpType.subtract,
                    )
                    nc.sync.dma_start(out=outp[w * P:(w + 1) * P, :], in_=ob[:])

    nc.compile()
    return nc


# ---------------------------------------------------------------- entry point

LAST_RESULT = {}


def _run(cfg, x, edge_index, W1, b1, W2, b2, trace=False):
    from concourse.bass_utils import run_bass_kernel_spmd

    sched, in_maps, perms = host_prep(cfg, x, edge_index, W1, b1, W2, b2)
    nc = build_program(cfg, sched)
    res = run_bass_kernel_spmd(
        nc, in_maps, list(range(N_CORES)), trace=trace,
    )
    LAST_RESULT["exec_time_ns"] = res.exec_time_ns
    LAST_RESULT["mean_exec_time_ns"] = res.mean_exec_time_ns
    N, NP, C = cfg["N"], cfg["NP"], cfg["C"]
    full = np.empty((N, C), dtype=np.float32)
    for k in range(N_CORES):
        outk = np.asarray(res.results[k]["out"], dtype=np.float32)
        blk = full[k * NP:(k + 1) * NP]
        blk[perms[k]] = outk[:NP]
    return full


def kernel(x, edge_index, W1, b1, W2, b2):
    trace = bool(int(os.environ.get("GNN_TRACE", "0")))
    return _run(FULL_CFG, x, edge_index, W1, b1, W2, b2, trace=trace)



# revision 17
# speedup vs baseline: 1.7396x; 1.0058x over previous
"""GNN message-passing v4: batched dma_gather + tile-level selection matmuls.

vs v2: SEG=128 (one selection matrix per column covering the whole 128-dst
tile -> minimal padding), per-tile accumulator tiles (no shared-tile hazard
serialization), gather ring depth 3.
"""

import os
import sys

sys.path.insert(0, "/opt/trn_rl_repo")

import numpy as np
import ml_dtypes

BF16 = ml_dtypes.bfloat16

N_CORES = 8
P = 128
CALLCOLS = 64     # 128-edge columns per dma_gather call (num_idxs 8192, HW-validated)
TW = 128          # table row width (bf16 elements) -> 256B rows


def _make_cfg(n_nodes, n_edges, f_in=512, hid=64, n_cls=40):
    np_ = n_nodes // N_CORES
    assert np_ * N_CORES == n_nodes
    nw = (np_ + P - 1) // P
    npad = nw * P
    n_chunks = min(4, nw)
    tiles = [nw // n_chunks + (1 if i < nw % n_chunks else 0) for i in range(n_chunks)]
    tstart = np.concatenate([[0], np.cumsum(tiles)]).astype(int)
    cs = [int(tstart[i] * P) for i in range(n_chunks + 1)]
    cz = [cs[i + 1] - cs[i] for i in range(n_chunks)]
    crows = [1 + N_CORES * z for z in cz]
    cb = np.concatenate([[0], np.cumsum(crows)]).astype(int)
    assert max(crows) <= 32767
    return dict(
        N=n_nodes, E=n_edges, F=f_in, H=hid, C=n_cls, NP=np_, NW=nw, NPAD=npad,
        NCH=n_chunks, TILES=tiles, TSTART=tstart, CS=cs, CZ=cz,
        CROWS=crows, CB=cb, TOT=int(cb[-1]),
    )


FULL_CFG = _make_cfg(100000, 3200000)


# ---------------------------------------------------------------- host prep

def host_prep(cfg, x, edge_index, W1, b1, W2, b2):
    N, NP, NW, NCH = cfg["N"], cfg["NP"], cfg["NW"], cfg["NCH"]
    CS, CZ, CB = cfg["CS"], cfg["CZ"], cfg["CB"]
    src = np.asarray(edge_index[0]).astype(np.int64)
    dst = np.asarray(edge_index[1]).astype(np.int64)

    r = np.arange(N, dtype=np.int64) // NP
    l = np.arange(N, dtype=np.int64) % NP
    csb = np.asarray(CS)
    ch_of = np.searchsorted(csb, l, side="right") - 1
    czv = np.asarray(CZ + [1])[ch_of]
    locrow = 1 + r * czv + (l - csb[ch_of])

    # per-core sorted edge views + per-(chunk, tile) counts
    views = []
    cnts = np.zeros((N_CORES, NCH, NW), dtype=np.int64)
    for k in range(N_CORES):
        sel = (dst >= k * NP) & (dst < (k + 1) * NP)
        s_k = src[sel]
        d_k = dst[sel] - k * NP
        key = ch_of[s_k] * (NP + 1) + d_k
        order = np.argsort(key, kind="stable")
        s_k, d_k = s_k[order], d_k[order]
        ch_k = ch_of[s_k]
        lr_k = locrow[s_k]
        views.append((d_k, ch_k, lr_k))
        for c in range(NCH):
            m = ch_k == c
            tt = d_k[m] // P
            bc = np.bincount(tt, minlength=NW)
            cnts[k, c, :] = bc

    # common column layout: per (chunk, tile) max over cores of ceil(cnt/128)
    ncol_ct = np.maximum(1, (cnts + P - 1) // P).max(axis=0)  # [NCH, NW]
    # meta per chunk: list of (tile, start, stop, tc_last)
    common_meta = []
    for c in range(NCH):
        meta_c = []
        for t in range(NW):
            nc_ = int(ncol_ct[c, t])
            for j in range(nc_):
                meta_c.append([t, j == 0, j == nc_ - 1, j == nc_ - 1])
        common_meta.append(meta_c)
    totcols = int(ncol_ct.sum())

    # per-core index streams + selection matrices on the common layout
    idxp_all, s_all = [], []
    for k in range(N_CORES):
        d_k, ch_k, lr_k = views[k]
        idx_list, s_list = [], []
        for c in range(NCH):
            m = ch_k == c
            dc = d_k[m]
            lrc = lr_k[m]
            tt = dc // P
            # edges already sorted by dst within chunk -> tiles contiguous
            tstarts = np.concatenate([[0], np.cumsum(np.bincount(tt, minlength=NW))])
            for t in range(NW):
                a, b = int(tstarts[t]), int(tstarts[t + 1])
                cnt = b - a
                nc_ = int(ncol_ct[c, t])
                rows = np.zeros(nc_ * P, dtype=np.int16)
                rows[:cnt] = lrc[a:b]
                dloc = np.full(nc_ * P, -1, dtype=np.int64)
                dloc[:cnt] = dc[a:b] - t * P
                for j in range(nc_):
                    sm = np.zeros((P, P), dtype=BF16)
                    dj = dloc[j * P:(j + 1) * P]
                    val = dj >= 0
                    sm[np.arange(P)[val], dj[val]] = 1.0
                    s_list.append(sm)
                    idx_list.append(rows[j * P:(j + 1) * P])
        st = np.concatenate(idx_list)
        t16 = st.reshape(totcols * 8, 16).T
        idxp_all.append(np.ascontiguousarray(np.tile(t16, (8, 1))))
        s_all.append(np.ascontiguousarray(
            np.stack(s_list, axis=1).reshape(P, -1)))

    W1b = np.asarray(W1, dtype=np.float32).astype(BF16)
    W2b = np.asarray(W2, dtype=np.float32).astype(BF16)
    b1r = np.tile(np.asarray(b1, dtype=np.float32)[None, :], (P, 1))
    b2a = np.asarray(b2, dtype=np.float32) - np.asarray(W2, np.float32).sum(0)
    b2r = np.tile(b2a[None, :], (P, 1))
    in_maps = []
    xf = np.asarray(x, dtype=np.float32)
    for k in range(N_CORES):
        xT = np.ascontiguousarray(xf[k * NP:(k + 1) * NP].T).astype(BF16)
        in_maps.append(dict(
            xT=xT, W1=W1b, b1r=b1r, W2=W2b, b2r=b2r,
            idxp=idxp_all[k], smat=s_all[k],
        ))
    sched = dict(common_meta=common_meta, totcols=totcols)
    return sched, in_maps


# ---------------------------------------------------------------- device code

def build_program(cfg, sched):
    import concourse.bass as bass
    import concourse.bacc as bacc
    import concourse.mybir as mybir
    from concourse.tile import TileContext
    from concourse.masks import make_identity

    dt = mybir.dt
    N, F, H, C = cfg["N"], cfg["F"], cfg["H"], cfg["C"]
    NP, NW, NPAD, NCH = cfg["NP"], cfg["NW"], cfg["NPAD"], cfg["NCH"]
    CS, CZ, CB, CROWS = cfg["CS"], cfg["CZ"], cfg["CB"], cfg["CROWS"]
    TOT = cfg["TOT"]
    meta = sched["common_meta"]
    ncols_ch = [len(m) for m in meta]
    totcols = sched["totcols"]
    KF = F // P

    nc = bacc.Bacc(
        "TRN2", target_bir_lowering=False, debug=False, num_devices=N_CORES
    )
    xT = nc.declare_dram_parameter("xT", [F, NP], dt.bfloat16, isOutput=False)
    W1p = nc.declare_dram_parameter("W1", [F, H], dt.bfloat16, isOutput=False)
    b1p = nc.declare_dram_parameter("b1r", [P, H], dt.float32, isOutput=False)
    W2p = nc.declare_dram_parameter("W2", [H, C], dt.bfloat16, isOutput=False)
    b2p = nc.declare_dram_parameter("b2r", [P, C], dt.float32, isOutput=False)
    ixp = nc.declare_dram_parameter("idxp", [P, totcols * 8], dt.int16, isOutput=False)
    smp = nc.declare_dram_parameter(
        "smat", [P, totcols * P], dt.bfloat16, isOutput=False
    )
    outp = nc.declare_dram_parameter("out", [NPAD, C], dt.float32, isOutput=True)

    rg = [list(range(N_CORES))]

    calls = []  # (chunk, global col start, ncols)
    goff = 0
    for c in range(NCH):
        o = 0
        while o < ncols_ch[c]:
            n = min(CALLCOLS, ncols_ch[c] - o)
            calls.append((c, goff + o, n))
            o += n
        goff += ncols_ch[c]

    # chunks contributing per tile (always NCH here since ncol>=1 everywhere)
    tile_nch = [NCH] * NW

    with TileContext(nc) as tc:
        with (
            tc.tile_pool(name="const", bufs=1) as const,
            tc.tile_pool(name="dram", bufs=1, space="DRAM") as dram,
            tc.tile_pool(name="xp", bufs=3) as xp,
            tc.tile_pool(name="hb", bufs=2) as hb,
            tc.tile_pool(name="ixpool", bufs=3) as ixpool,
            tc.tile_pool(name="smpool", bufs=3) as smpool,
            tc.tile_pool(name="gpool", bufs=3) as gpool,
            tc.tile_pool(name="acc", bufs=1) as accp,
            tc.tile_pool(name="sp", bufs=4) as sp,
            tc.tile_pool(name="ps", bufs=2, space="PSUM") as ps,
        ):
            w1sb = const.tile([P, KF, H], dt.bfloat16)
            nc.sync.dma_start(out=w1sb[:], in_=W1p[:].rearrange("(c p) h -> p c h", p=P))
            w2sb = const.tile([H, C], dt.bfloat16)
            nc.sync.dma_start(out=w2sb[:], in_=W2p[:])
            b1sb = const.tile([P, H], dt.float32)
            nc.sync.dma_start(out=b1sb[:], in_=b1p[:])
            b2sb = const.tile([P, C], dt.float32)
            nc.sync.dma_start(out=b2sb[:], in_=b2p[:])
            ident = const.tile([P, P], dt.bfloat16)
            make_identity(nc, ident[:])

            h1k = dram.tile([NPAD, TW], dt.bfloat16)
            t2k = dram.tile([NPAD, TW], dt.bfloat16)
            tb1 = dram.tile([TOT, TW], dt.bfloat16)
            tb2 = dram.tile([TOT, TW], dt.bfloat16)

            zt = const.tile([1, TW], dt.bfloat16)
            nc.gpsimd.memset(zt[:], 0.0)
            for c in range(NCH):
                nc.sync.dma_start(out=tb1[CB[c]:CB[c] + 1, :], in_=zt[:])
                nc.sync.dma_start(out=tb2[CB[c]:CB[c] + 1, :], in_=zt[:])

            # per-tile accumulators (separate tiles -> no hazard serialization)
            acc1 = [accp.tile([P, H], dt.float32, name=f"acc1_{t}") for t in range(NW)]
            acc2 = [accp.tile([P, C], dt.float32, name=f"acc2_{t}") for t in range(NW)]

            # --- phase 1: h1 = x@W1 + b1, chunked AllGather
            xTr = xT[:].rearrange("(c p) n -> p c n", p=P)
            for c in range(NCH):
                t0, t1 = int(cfg["TSTART"][c]), int(cfg["TSTART"][c + 1])
                ntl = t1 - t0
                h1b = hb.tile([P, ntl, TW], dt.bfloat16, tag="h1b", name=f"h1b_{c}")
                nc.vector.memset(h1b[:].rearrange("p a b -> p (a b)"), 0.0)
                for i, nt in enumerate(range(t0, t1)):
                    cs_ = min(P, NP - nt * P)
                    if cs_ <= 0:
                        continue
                    xt = xp.tile([P, KF, P], dt.bfloat16, tag="xt")
                    nc.sync.dma_start(out=xt[:, :, :cs_], in_=xTr[:, :, nt * P:nt * P + cs_])
                    ph = ps.tile([P, H], dt.float32, tag="ph", bufs=2)
                    for kf in range(KF):
                        nc.tensor.matmul(
                            out=ph[:cs_, :], lhsT=xt[:, kf, :cs_], rhs=w1sb[:, kf, :],
                            start=(kf == 0), stop=(kf == KF - 1),
                        )
                    nc.vector.tensor_tensor(
                        out=h1b[:cs_, i, :H], in0=ph[:cs_, :], in1=b1sb[:cs_, :],
                        op=mybir.AluOpType.add,
                    )
                nc.sync.dma_start(
                    out=h1k[CS[c]:CS[c] + ntl * P, :].rearrange(
                        "(a p) w -> p a w", p=P),
                    in_=h1b[:],
                )
                nc.gpsimd.collective_compute(
                    "AllGather", mybir.AluOpType.bypass, replica_groups=rg,
                    ins=[h1k[CS[c]:CS[c] + CZ[c], :]],
                    outs=[tb1[CB[c] + 1:CB[c] + 1 + N_CORES * CZ[c], :]],
                )

            def agg_pass(tbl, width, acc, post_tile):
                pend = {}
                first = {}
                for (c, g0, ncols) in calls:
                    NI = ncols * P
                    ixt = ixpool.tile([P, ncols * 8], dt.int16, tag="ix")
                    nc.sync.dma_start(out=ixt[:], in_=ixp[:, g0 * 8:(g0 + ncols) * 8])
                    smt = smpool.tile([P, ncols, P], dt.bfloat16, tag="sm")
                    nc.sync.dma_start(
                        out=smt[:],
                        in_=smp[:, g0 * P:(g0 + ncols) * P].rearrange(
                            "p (n s) -> p n s", s=P),
                    )
                    gt = gpool.tile([P, ncols, TW], dt.bfloat16, tag="gt")
                    nc.gpsimd.dma_gather(
                        gt[:], tbl[CB[c]:CB[c] + CROWS[c], :], ixt[:], NI, NI, TW,
                        single_packet=False,
                    )
                    base = sum(ncols_ch[:c])
                    for j in range(ncols):
                        t, st, sp_, tc_last = meta[c][g0 - base + j]
                        if t not in pend:
                            pend[t] = ps.tile(
                                [P, width], dt.float32, tag="agg", bufs=2,
                                name=f"agg_{c}_{t}",
                            )
                        nc.tensor.matmul(
                            out=pend[t][:], lhsT=smt[:, j, :], rhs=gt[:, j, :width],
                            start=st, stop=sp_,
                        )
                        if tc_last:
                            pt = pend.pop(t)
                            if t not in first:
                                first[t] = 1
                                nc.vector.tensor_copy(out=acc[t][:], in_=pt[:])
                            else:
                                first[t] += 1
                                nc.vector.tensor_tensor(
                                    out=acc[t][:], in0=acc[t][:], in1=pt[:],
                                    op=mybir.AluOpType.add,
                                )
                            if first[t] == tile_nch[t]:
                                post_tile(t)

            # --- phase 2: L1 aggregate -> elu' -> t2 rows -> AllGather#2
            t2bufs = {}
            t2done = {}

            def make_t2(t):
                c = int(np.searchsorted(cfg["TSTART"], t, side="right") - 1)
                t0, t1 = int(cfg["TSTART"][c]), int(cfg["TSTART"][c + 1])
                if c not in t2bufs:
                    t2bufs[c] = hb.tile(
                        [P, t1 - t0, TW], dt.bfloat16, tag="t2b", name=f"t2b_{c}",
                    )
                    t2done[c] = 0
                    nc.vector.memset(t2bufs[c][:].rearrange("p a b -> p (a b)"), 0.0)
                red = acc1[t][:]
                m = sp.tile([P, H], dt.float32, tag="m")
                nc.vector.tensor_scalar_min(out=m[:], in0=red, scalar1=0.0)
                e = sp.tile([P, H], dt.float32, tag="e")
                nc.scalar.activation(e[:], m[:], mybir.ActivationFunctionType.Exp)
                gpr = sp.tile([P, H], dt.bfloat16, tag="gpr")
                nc.vector.scalar_tensor_tensor(
                    out=gpr[:], in0=red, scalar=0.0, in1=e[:],
                    op0=mybir.AluOpType.max, op1=mybir.AluOpType.add,
                )
                tr = ps.tile([H, P], dt.bfloat16, tag="tr", bufs=2)
                nc.tensor.transpose(out=tr[:], in_=gpr[:], identity=ident[:])
                trsb = sp.tile([H, P], dt.bfloat16, tag="trsb")
                nc.vector.tensor_copy(out=trsb[:], in_=tr[:])
                t2p = ps.tile([P, C], dt.float32, tag="t2p", bufs=2)
                nc.tensor.matmul(out=t2p[:], lhsT=trsb[:], rhs=w2sb[:, :C],
                                 start=True, stop=True)
                nc.vector.tensor_tensor(
                    out=t2bufs[c][:, t - t0, :C], in0=t2p[:], in1=b2sb[:, :C],
                    op=mybir.AluOpType.add,
                )
                t2done[c] += 1
                if t2done[c] == t1 - t0:
                    nc.sync.dma_start(
                        out=t2k[CS[c]:CS[c] + (t1 - t0) * P, :].rearrange(
                            "(a p) w -> p a w", p=P),
                        in_=t2bufs[c][:],
                    )
                    nc.gpsimd.collective_compute(
                        "AllGather", mybir.AluOpType.bypass, replica_groups=rg,
                        ins=[t2k[CS[c]:CS[c] + CZ[c], :]],
                        outs=[tb2[CB[c] + 1:CB[c] + 1 + N_CORES * CZ[c], :]],
                    )

            agg_pass(tb1, H, acc1, make_t2)

            # --- phase 3: L2 aggregate -> log_softmax (all per-tile tiles)
            def softmax_t(t):
                red = acc2[t][:]
                nm = sp.tile([P, 1], dt.float32, tag="nm")
                nc.vector.tensor_reduce(
                    out=nm[:], in_=red, axis=mybir.AxisListType.X,
                    op=mybir.AluOpType.max, negate=True,
                )
                sc = sp.tile([P, C], dt.float32, tag="sc")
                ssum = sp.tile([P, 1], dt.float32, tag="ssum")
                nc.scalar.activation(
                    sc[:], red, mybir.ActivationFunctionType.Exp,
                    bias=nm[:], accum_out=ssum[:],
                )
                ls = sp.tile([P, 1], dt.float32, tag="ls")
                nc.scalar.activation(ls[:], ssum[:], mybir.ActivationFunctionType.Ln)
                nc.vector.tensor_scalar(
                    out=acc2[t][:], in0=red, scalar1=nm[:], scalar2=ls[:],
                    op0=mybir.AluOpType.add, op1=mybir.AluOpType.subtract,
                )
                nc.sync.dma_start(
                    out=outp[t * P:(t + 1) * P, :], in_=acc2[t][:],
                )

            agg_pass(tb2, C, acc2, softmax_t)

    nc.compile()
    return nc


# ---------------------------------------------------------------- entry point

LAST_RESULT = {}


def _run(cfg, x, edge_index, W1, b1, W2, b2, trace=False):
    from concourse.bass_utils import run_bass_kernel_spmd

    sched, in_maps = host_prep(cfg, x, edge_index, W1, b1, W2, b2)
    nc = build_program(cfg, sched)
    res = run_bass_kernel_spmd(
        nc, in_maps, list(range(N_CORES)), trace=trace,
    )
    LAST_RESULT["exec_time_ns"] = res.exec_time_ns
    LAST_RESULT["mean_exec_time_ns"] = res.mean_exec_time_ns
    N, NP, C = cfg["N"], cfg["NP"], cfg["C"]
    full = np.empty((N, C), dtype=np.float32)
    for k in range(N_CORES):
        outk = np.asarray(res.results[k]["out"], dtype=np.float32)
        full[k * NP:(k + 1) * NP] = outk[:NP]
    return full


def kernel(x, edge_index, W1, b1, W2, b2):
    trace = bool(int(os.environ.get("GNN_TRACE", "0")))
    return _run(FULL_CFG, x, edge_index, W1, b1, W2, b2, trace=trace)
